# revision 37
# baseline (speedup 1.0000x reference)
"""Nearest-class-mean scores kernel for 8 Trainium2 NeuronCores.

Math (per the oracle):
    scores[m, n] = -(||X[m]||^2 + ||muK[n]||^2 - 2 X[m].muK[n])
    rowmin[m]    = min_n scores[m, n] - 1
    out[m, n]    = rowmin[m] if cK[n] == 0 else scores[m, n]

Strategy: data-parallel over the batch axis M (16384 -> 2048 rows/core),
muK/cK replicated. Per core, one fp32r GEMM with an augmented contraction
(K = 512 + 2 rows carrying -||x||^2 and -||mu||^2) writes the final scores
straight into PSUM. The epilogue is a reduce_min and one fused
out = min(scores, rowmin - 1 + BIG*visited[n]) tensor op, which applies the
not-visited mask with no select.

Host-side prep (cheap, numpy): X^T layout for the stationary operand,
row norms, the augmented rows, and the broadcast mask tile.
"""

import numpy as np


def _ensure_concourse():
    try:
        import concourse.bass  # noqa: F401
    except ImportError:
        import sys

        for p in ("/opt/trn_rl_repo", "/root/.axon_site/_ro/trn_rl_repo"):
            if p not in sys.path:
                sys.path.insert(0, p)


M, N, D = 16384, 1000, 512
NCORES = 8
M_LOC = M // NCORES  # 2048
KT = D // 128  # 4 full contraction chunks
KA = 2  # augmented rows: [-xsq | 1] x [1 | -musq]
BIG = 1.0e30

_NC_CACHE = {}


def _build_nc(
    m_loc=M_LOC,
    debug=False,
    no_pe=False,
    no_epi=False,
    no_store=False,
    epi="perm",  # "perm": visited-cols-first layout; "dve": min-trick epilogue
    slice_loads=False,
    repeat=1,
    loop_n=None,  # if set, wrap body (incl. loads) in tc.For_i(0, loop_n) for benching
    data_dt="bf16",  # dtype of the 512 data rows: "bf16" or "f32r"
    n_vis=N,  # number of visited (unmasked) columns; columns [n_vis:] get rowmin-1
    store_eng="sync",  # engine for output stores: "sync" or "gpsimd"
):
    from concourse import bacc, tile
    import concourse.mybir as mybir

    f32 = mybir.dt.float32
    f32r = mybir.dt.float32r
    ddt = mybir.dt.bfloat16 if data_dt == "bf16" else f32r
    Alu = mybir.AluOpType
    Act = mybir.ActivationFunctionType

    mt_cnt = m_loc // 128

    nc = bacc.Bacc("TRN2", target_bir_lowering=False, debug=debug, num_devices=NCORES)

    xt = nc.dram_tensor("xt", (D, m_loc), ddt, kind="ExternalInput").ap()
    bt = nc.dram_tensor("bt", (D, N), ddt, kind="ExternalInput").ap()
    xaug = nc.dram_tensor("xaug", (KA, m_loc), f32r, kind="ExternalInput").ap()
    baug = nc.dram_tensor("baug", (KA, N), f32r, kind="ExternalInput").ap()
    if epi == "dve":
        bigv_bc = nc.dram_tensor("bigv_bc", (128, N), f32, kind="ExternalInput").ap()
    out = nc.dram_tensor("out", (m_loc, N), f32, kind="ExternalOutput").ap()

    with tile.TileContext(nc) as tc:
        with (
            tc.tile_pool(name="a", bufs=1) as a_pool,
            tc.tile_pool(name="b", bufs=1) as b_pool,
            tc.tile_pool(name="cst", bufs=1) as c_pool,
            tc.tile_pool(name="s", bufs=4) as s_pool,
            tc.tile_pool(name="u", bufs=3) as u_pool,
            tc.tile_pool(name="fin", bufs=4) as f_pool,
            tc.tile_pool(name="acc", bufs=4) as acc_pool,
            tc.tile_pool(name="psum", bufs=4, space="PSUM") as p_pool,
        ):
            def emit_body():
                # b chunks first (every m-tile needs all of bt)
                b_tiles = []
                for k in range(KT):
                    t = b_pool.tile([128, N], ddt, tag=f"b{k}", name=f"b{k}")
                    nc.sync.dma_start(t[:], bt[128 * k : 128 * (k + 1), :])
                    b_tiles.append(t)
                b_aug = b_pool.tile([KA, N], f32r, tag="b_aug", name="b_aug")
                nc.sync.dma_start(b_aug[:], baug[:, :])

                if epi == "dve":
                    bigv_t = c_pool.tile([128, N], f32, tag="bigv", name="bigv")
                    nc.sync.dma_start(bigv_t[:], bigv_bc[:, :])
                elif epi == "perm" and n_vis < N:
                    ones_t = c_pool.tile([128, 1], f32, tag="ones", name="ones")
                    nc.vector.memset(ones_t[:], 1.0)

                # a chunks, loaded in m-slices so the first m-tiles' weights
                # land before the whole payload is in
                a_tiles = [
                    a_pool.tile([128, m_loc], ddt, tag=f"a{k}", name=f"a{k}")
                    for k in range(KT)
                ]
                a_aug = a_pool.tile([KA, m_loc], f32r, tag="a_aug", name="a_aug")
                if slice_loads:
                    n_sl = 4
                    sl_w = m_loc // n_sl
                    for s in range(n_sl):
                        msl = slice(s * sl_w, (s + 1) * sl_w)
                        for k in range(KT):
                            nc.sync.dma_start(
                                a_tiles[k][:, msl], xt[128 * k : 128 * (k + 1), msl]
                            )
                        nc.sync.dma_start(a_aug[:, msl], xaug[:, msl])
                else:
                    for k in range(KT):
                        nc.sync.dma_start(a_tiles[k][:], xt[128 * k : 128 * (k + 1), :])
                    nc.sync.dma_start(a_aug[:], xaug[:, :])

                halves = [(0, 512), (512, 488)]
                a_all = a_tiles + [a_aug]
                b_all = b_tiles + [b_aug]
                for mt in [t for _ in range(repeat) for t in range(mt_cnt)]:
                    ps = p_pool.tile([128, 1024], f32, tag="ps", name="ps")
                    if no_pe:
                        nc.vector.memset(ps[:], 0.0)
                    else:
                        for off, w in halves:
                            for k in range(KT + 1):
                                nc.tensor.matmul(
                                    ps[:, off : off + w],
                                    lhsT=a_all[k][:, 128 * mt : 128 * (mt + 1)],
                                    rhs=b_all[k][:, off : off + w],
                                    start=(k == 0),
                                    stop=(k == KT),
                                )
                    fin = f_pool.tile([128, N], f32, tag="fin", name="fin")
                    if no_epi:
                        nc.vector.tensor_copy(fin[:], ps[:, 0:N])
                    elif epi == "perm":
                        # rowmin over ALL columns (mask not applied yet)
                        acc = acc_pool.tile([128, 1], f32, tag="acc", name="acc")
                        nc.vector.tensor_reduce(
                            out=acc[:], in_=ps[:, 0:N], axis=mybir.AxisListType.X, op=Alu.min
                        )
                        if n_vis > 0:
                            # visited region: straight copy of scores
                            nc.vector.tensor_copy(fin[:, 0:n_vis], ps[:, 0:n_vis])
                        if n_vis < N:
                            # not-visited region: rowmin - 1 broadcast
                            nc.vector.tensor_scalar_add(
                                fin[:, n_vis:N],
                                acc[:, 0:1].to_broadcast([128, N - n_vis]),
                                -1.0,
                            )
                    elif epi == "dve":
                        acc = acc_pool.tile([128, 1], f32, tag="acc", name="acc")
                        nc.vector.tensor_reduce(
                            out=acc[:], in_=ps[:, 0:N], axis=mybir.AxisListType.X, op=Alu.min
                        )
                        u_sb = u_pool.tile([128, N], f32, tag="u", name="u")
                        nc.scalar.activation(
                            u_sb[:], bigv_t[:], Act.Identity, bias=acc[:, 0:1], scale=1.0
                        )
                        nc.vector.tensor_tensor(
                            out=fin[:], in0=ps[:, 0:N], in1=u_sb[:], op=Alu.min
                        )
                    else:
                        raise ValueError(epi)
                    if not no_store:
                        st = nc.gpsimd if store_eng == "gpsimd" else nc.sync
                        st.dma_start(out[128 * mt : 128 * (mt + 1), :], fin[:])

            if loop_n is not None:
                with tc.For_i(0, loop_n, 1):
                    emit_body()
            else:
                emit_body()

    nc.compile()
    return nc


def _get_nc(n_vis):
    if n_vis not in _NC_CACHE:
        _NC_CACHE[n_vis] = _build_nc(epi="perm", n_vis=n_vis)
    return _NC_CACHE[n_vis]


def _prep_host(X, muK, cK, data_dt="bf16", perm=None):
    import ml_dtypes

    dnp = ml_dtypes.bfloat16 if data_dt == "bf16" else np.float32

    X = np.ascontiguousarray(X, dtype=np.float32)
    muK = np.ascontiguousarray(muK, dtype=np.float32)
    cK = np.asarray(cK)
    if perm is not None:
        muK = muK[perm]
        cK = cK[perm]

    xt = np.ascontiguousarray(X.T.astype(dnp))  # (D, M)
    bt = np.ascontiguousarray((muK.T * np.float32(2.0)).astype(dnp))  # (D, N)

    xsq = np.einsum("md,md->m", X, X).astype(np.float32)
    musq = np.einsum("nd,nd->n", muK, muK).astype(np.float32)

    xaug = np.empty((KA, X.shape[0]), dtype=np.float32)
    xaug[0] = -xsq
    xaug[1] = 1.0

    baug = np.empty((KA, N), dtype=np.float32)
    baug[0] = 1.0
    baug[1] = -musq

    vis = (cK != 0).astype(np.float32)
    bigv = (vis * np.float32(BIG) - np.float32(1.0)).astype(np.float32)
    bigv_bc = np.ascontiguousarray(np.broadcast_to(bigv, (128, N)))
    return xt, bt, xaug, baug, bigv_bc


def kernel(X, muK, cK, _trace=False, _tmpdir=None):
    _ensure_concourse()
    from concourse import bass_utils

    cK = np.asarray(cK)
    # visited classes first so the not-visited mask is a contiguous column
    # range on-device; undone on the output below
    perm = np.argsort(cK == 0, kind="stable")
    n_vis = int((cK != 0).sum())

    nc = _get_nc(n_vis)
    xt, bt, xaug, baug, _ = _prep_host(X, muK, cK, perm=perm)
    in_maps = []
    for c in range(NCORES):
        sl = slice(c * M_LOC, (c + 1) * M_LOC)
        in_maps.append(
            {
                "xt": np.ascontiguousarray(xt[:, sl]),
                "bt": bt,
                "xaug": np.ascontiguousarray(xaug[:, sl]),
                "baug": baug,
            }
        )
    res = bass_utils.run_bass_kernel_spmd(
        nc,
        in_maps,
        core_ids=list(range(NCORES)),
        trace=_trace,
        tmpdir=_tmpdir,
    )
    out_p = np.concatenate([r["out"] for r in res.results], axis=0)
    inv = np.empty_like(perm)
    inv[perm] = np.arange(N)
    out = np.ascontiguousarray(out_p[:, inv])
    if _trace:
        kernel._last_results = res
    return out
